# revision 25
# baseline (speedup 1.0000x reference)
"""Multi-head self-attention with RoPE on 8 Trainium2 NeuronCores.

Tensor-parallel over heads: core c owns heads 2c, 2c+1.  Each core computes
Q/K/V projections for its heads (full sequence), causal attention in a
transposed layout, and a partial out-projection against its 128-column slice
of Wo.  The host sums the 8 partial outputs.

Matmuls run in bf16 (fp32 PSUM accumulation): bf16 gets fast weight load and
weight-load/matmul overlap on the PE, which fp32/fp32r do not.

Per-core dataflow (S=4096, D=1024, hd=64):
  xT [1024,4096](bf16) --PE--> qt/kt [128,4096] via RoPE, vt [128,4096]
  RoPE: qt = ps*cs + swap32(ps*sn_signed); the 32-row swap runs on idle DMA
  engines, channel order pre-permuted on the host ([evens|odds] per head)
  vt --PE transpose--> vaug[h] [128, 32*65] bf16 (V natural layout + ones
  column per 128-k-chunk; the ones column is the softmax denominator)
  scores^T [k,q] = kt_h^T @ qt_h (K=64) in PSUM; causal diagonal tiles get a
  [128,128] additive band mask; exp on ScalarE (scale=1/8 folded in) writes
  bf16 probs, skipping fully-masked columns
  attn@V: vaug [128,65] x probs [128,512] accumulated over k-chunks in PSUM
  normalize: approx reciprocal of row 64, PE K=1 broadcast, DVE multiply
  out_proj: outT [128,128] x woT [128,512], DVE evict, DMA to DRAM
"""

from contextlib import ExitStack

import ml_dtypes
import numpy as np

import concourse.bass as bass
import concourse.mybir as mybir
import concourse.tile as tile
from concourse import bacc
from concourse.masks import make_identity

F32 = mybir.dt.float32
BF16 = mybir.dt.bfloat16
EXPF = mybir.ActivationFunctionType.Exp
COPYF = mybir.ActivationFunctionType.Copy

D_MODEL = 1024
HEAD_DIM = 64
HALF = 32
THETA = 10000.0
N_CORES = 8
HPC = 2  # heads per core


# ---------------------------------------------------------------------------
# host-side sharding
# ---------------------------------------------------------------------------

def _perm128():
    # within each head's 64 rows: evens then odds, so RoPE pairs are p <-> p+-32
    g = []
    for h in range(HPC):
        for par in (0, 1):
            g.extend(h * HEAD_DIM + 2 * i + par for i in range(HALF))
    return np.array(g, dtype=np.int64)


def _rope_tables(s_len):
    # qt = ps*cs + swap32(ps*sn), where swap32 exchanges rows p <-> p+-32
    # within each 64-row head block.  Row p of sn carries the sign that the
    # OUTPUT row swap(p) needs: +sin for p%64<32, -sin otherwise.
    inv_freq = 1.0 / (THETA ** (np.arange(HALF, dtype=np.float64) * 2.0 / HEAD_DIM))
    ang = np.arange(s_len, dtype=np.float64)[None, :] * inv_freq[:, None]
    cs = np.tile(np.cos(ang), (4, 1)).astype(np.float32)
    sinv = np.sin(ang)
    sn = np.concatenate([sinv, -sinv, sinv, -sinv], axis=0).astype(np.float32)
    return cs, sn


def _band_mask():
    # additive diagonal band: band[kk, b] = 0 if b >= kk else -240
    kk = np.arange(128)[:, None]
    b = np.arange(128)[None, :]
    return np.where(b >= kk, 0.0, -240.0).astype(np.float32)


def _bf16(a):
    return np.asarray(a, np.float32).astype(ml_dtypes.bfloat16)


def shard_inputs(x, Wq, Wk, Wv, Wo):
    x2 = np.ascontiguousarray(np.asarray(x, np.float32).reshape(-1, D_MODEL))
    s_len = x2.shape[0]
    xT = np.ascontiguousarray(_bf16(x2.T))
    cs, sn = _rope_tables(s_len)
    band = _band_mask()
    g = _perm128()
    Wq, Wk, Wv, Wo = (np.asarray(w, np.float32) for w in (Wq, Wk, Wv, Wo))
    per_core = []
    for c in range(N_CORES):
        rows = slice(c * 128, (c + 1) * 128)
        per_core.append({
            "xT": xT,
            "wqT": np.ascontiguousarray(_bf16(Wq[rows][g].T)),
            "wkT": np.ascontiguousarray(_bf16(Wk[rows][g].T)),
            "wvT": np.ascontiguousarray(_bf16(Wv[rows].T)),
            "woT": np.ascontiguousarray(_bf16(Wo[:, rows].T)),
            "cs": _bf16(cs),
            "sn": _bf16(sn),
            "band": band,
        })
    return per_core


# ---------------------------------------------------------------------------
# device kernel
# ---------------------------------------------------------------------------

def build_program(s_len):
    """Build the SPMD Bass program for one core (same program, all cores)."""
    n_sc = s_len // 512   # 512-wide s/q chunks
    n_kc = s_len // 128   # 128-wide k chunks
    n_dc = D_MODEL // 128

    nc = bacc.Bacc("TRN2", target_bir_lowering=False, debug=False)

    xT = nc.dram_tensor("xT", [D_MODEL, s_len], BF16, kind="ExternalInput").ap()
    wqT = nc.dram_tensor("wqT", [D_MODEL, 128], BF16, kind="ExternalInput").ap()
    wkT = nc.dram_tensor("wkT", [D_MODEL, 128], BF16, kind="ExternalInput").ap()
    wvT = nc.dram_tensor("wvT", [D_MODEL, 128], BF16, kind="ExternalInput").ap()
    woT = nc.dram_tensor("woT", [128, D_MODEL], BF16, kind="ExternalInput").ap()
    cs_d = nc.dram_tensor("cs", [128, s_len], BF16, kind="ExternalInput").ap()
    sn_d = nc.dram_tensor("sn", [128, s_len], BF16, kind="ExternalInput").ap()
    band_d = nc.dram_tensor("band", [128, 128], F32, kind="ExternalInput").ap()
    y = nc.dram_tensor("y", [s_len, D_MODEL], F32, kind="ExternalOutput").ap()

    with tile.TileContext(nc) as tc, ExitStack() as ctx:
        const = ctx.enter_context(tc.tile_pool(name="const", bufs=1))
        resid = ctx.enter_context(tc.tile_pool(name="resid", bufs=1))
        xin = ctx.enter_context(tc.tile_pool(name="xin", bufs=24))
        stage = ctx.enter_context(tc.tile_pool(name="stage", bufs=4))
        probs_p = ctx.enter_context(tc.tile_pool(name="probs", bufs=8))
        ysb_p = ctx.enter_context(tc.tile_pool(name="ysb", bufs=4))
        smalls = ctx.enter_context(tc.tile_pool(name="smalls", bufs=2))

        # ---- constants / weights in SBUF ----
        wq_sb = const.tile([128, D_MODEL], BF16, tag="wq")
        wk_sb = const.tile([128, D_MODEL], BF16, tag="wk")
        wv_sb = const.tile([128, D_MODEL], BF16, tag="wv")
        for d in range(n_dc):
            sl = slice(d * 128, (d + 1) * 128)
            nc.gpsimd.dma_start(wq_sb[:, sl], wqT[sl, :])
            nc.gpsimd.dma_start(wk_sb[:, sl], wkT[sl, :])
            nc.gpsimd.dma_start(wv_sb[:, sl], wvT[sl, :])
        wo_sb = const.tile([128, D_MODEL], BF16, tag="wo")
        nc.gpsimd.dma_start(wo_sb[:], woT[:])
        cs_sb = const.tile([128, s_len], BF16, tag="cs")
        sn_sb = const.tile([128, s_len], BF16, tag="sn")
        nc.gpsimd.dma_start(cs_sb[:], cs_d[:])
        nc.gpsimd.dma_start(sn_sb[:], sn_d[:])
        band_sb = const.tile([128, 128], F32, tag="band")
        nc.gpsimd.dma_start(band_sb[:], band_d[:])
        ident = const.tile([128, 128], BF16, tag="ident")
        make_identity(nc, ident[:])
        ones_sb = const.tile([65, 128], BF16, tag="ones")
        nc.vector.memset(ones_sb[:], 1.0)
        zbias = const.tile([128, 1], F32, tag="zbias")
        nc.vector.memset(zbias[:], 0.0)

        # ---- persistent activations ----
        qt = resid.tile([128, s_len], BF16, tag="qt")
        kt = resid.tile([128, s_len], BF16, tag="kt")
        vt = resid.tile([128, s_len], BF16, tag="vt")
        vaug = [resid.tile([128, n_kc * 65], BF16, tag=f"vaug{h}", name=f"vaug{h}")
                for h in range(HPC)]
        outT = resid.tile([128, s_len], BF16, tag="outT")
        for h in range(HPC):
            ones_cols = vaug[h].rearrange("p (k c) -> p k c", c=65)[:, :, 64:65]
            nc.vector.memset(ones_cols, 1.0)

        # ================= phase A: QKV + RoPE + V transpose =================
        with tc.tile_pool(name="psA", bufs=2, space="PSUM") as psA:
            for j in range(n_sc):
                jsl = slice(j * 512, (j + 1) * 512)
                xts = []
                for d in range(n_dc):
                    xt = xin.tile([128, 512], BF16, tag="xt")
                    nc.sync.dma_start(xt[:], xT[d * 128:(d + 1) * 128, jsl])
                    xts.append(xt)

                ps_q = psA.tile([128, 512], F32, tag="q")
                ps_k = psA.tile([128, 512], F32, tag="k")
                ps_v = psA.tile([128, 512], F32, tag="v")
                for w_sb, ps in ((wq_sb, ps_q), (wk_sb, ps_k), (wv_sb, ps_v)):
                    for d in range(n_dc):
                        nc.tensor.matmul(
                            ps[:],
                            w_sb[:, d * 128:(d + 1) * 128],
                            xts[d][:],
                            start=(d == 0),
                            stop=(d == n_dc - 1),
                        )

                # RoPE: qt = ps*cs + swap32(ps*sn_signed).  The 32-row swap
                # runs on idle DMA engines (SBUF->SBUF); the final add sees
                # base-partition-matched SBUF operands (a DVE requirement).
                for ps, dst in ((ps_q, qt), (ps_k, kt)):
                    P = stage.tile([128, 512], F32, tag="P")
                    Rr = stage.tile([128, 512], F32, tag="Rr")
                    Rs = stage.tile([128, 512], F32, tag="Rs")
                    nc.vector.tensor_mul(P[:], ps[:], cs_sb[:, jsl])
                    nc.vector.tensor_mul(Rr[:], ps[:], sn_sb[:, jsl])
                    for q in range(4):
                        src = q + 1 if q % 2 == 0 else q - 1
                        nc.sync.dma_start(Rs[q * 32:(q + 1) * 32, :],
                                          Rr[src * 32:(src + 1) * 32, :])
                    nc.vector.tensor_add(dst[:, jsl], P[:], Rs[:])
                nc.scalar.copy(vt[:, jsl], ps_v[:])

                # V transpose: 4 k-chunks per s-chunk, natural layout + ones col
                for kk in range(4):
                    kc = 4 * j + kk
                    pt = psA.tile([128, 128], BF16, tag="pt")
                    nc.tensor.transpose(
                        pt[:], vt[:, kc * 128:(kc + 1) * 128], ident[:]
                    )
                    for h in range(HPC):
                        nc.vector.tensor_copy(
                            vaug[h][:, kc * 65:kc * 65 + 64],
                            pt[:, h * 64:(h + 1) * 64],
                        )

        # ================= phase B: attention + out_proj =================
        with (
            tc.tile_pool(name="psS", bufs=2, space="PSUM") as psS,
            tc.tile_pool(name="psO", bufs=1, space="PSUM") as psO,
            tc.tile_pool(name="psY", bufs=1, space="PSUM") as psY,
        ):
            for qc in range(n_sc):
                qs = qc * 512
                qsl = slice(qs, qs + 512)
                nk = 4 * (qc + 1)
                o_ps = [psO.tile([65, 512], F32, tag=f"o{h}", name=f"o{h}")
                        for h in range(HPC)]
                for pair in range(nk // 2):
                    kcs = (2 * pair, 2 * pair + 1)
                    for h in range(HPC):
                        rb = 64 * h
                        sc_ps = psS.tile([128, 1024], F32, tag="sc")
                        for side, kc in enumerate(kcs):
                            ssl = slice(side * 512, side * 512 + 512)
                            nc.tensor.matmul(
                                sc_ps[:, ssl],
                                kt[rb:rb + 64, kc * 128:(kc + 1) * 128],
                                qt[rb:rb + 64, qsl],
                            )
                        pr = probs_p.tile([128, 1024], BF16, tag="pr")
                        diag = [kc * 128 - qs for kc in kcs]
                        if diag[1] < 0:
                            # fully-valid pair: one wide exp
                            nc.scalar.activation(pr[:], sc_ps[:], EXPF,
                                                 bias=zbias[:], scale=0.125)
                        else:
                            for side, kc in enumerate(kcs):
                                r = diag[side]
                                c0 = side * 512
                                if r < 0:
                                    nc.scalar.activation(
                                        pr[:, c0:c0 + 512],
                                        sc_ps[:, c0:c0 + 512],
                                        EXPF, bias=zbias[:], scale=0.125)
                                    continue
                                # columns < r are fully masked; the 128-wide
                                # band at [r, r+128) gets the additive mask
                                nc.vector.tensor_add(
                                    sc_ps[:, c0 + r:c0 + r + 128],
                                    sc_ps[:, c0 + r:c0 + r + 128],
                                    band_sb[:],
                                )
                                if r > 0:
                                    nc.vector.memset(pr[:, c0:c0 + r], 0.0)
                                nc.scalar.activation(
                                    pr[:, c0 + r:c0 + 512],
                                    sc_ps[:, c0 + r:c0 + 512],
                                    EXPF, bias=zbias[:], scale=0.125)
                        for side, kc in enumerate(kcs):
                            ssl = slice(side * 512, side * 512 + 512)
                            nc.tensor.matmul(
                                o_ps[h][:],
                                vaug[h][:, kc * 65:kc * 65 + 65],
                                pr[:, ssl],
                                start=(kc == 0),
                                stop=(kc == nk - 1),
                            )

                # normalize: outT[h] = o_ps[h][0:64] * (1/denom) broadcast.
                # Evict the accumulator to SBUF with two fast copies first so
                # the PSUM bank frees in ~1.3us; the reciprocal/broadcast
                # chain then runs off the critical path (its latency was
                # stalling the next q-chunk's attn@V and re-throttling HAM).
                onum = smalls.tile([128, 512], F32, tag="onum")
                bc_ps = psY.tile([128, 512], F32, tag="bc")
                for h in range(HPC):
                    rec = smalls.tile([65, 512], F32, tag="rec",
                                      name=f"rec{h}")
                    recb = smalls.tile([65, 512], BF16, tag="recb",
                                       name=f"recb{h}")
                    rs4 = smalls.tile([128, 4], F32, tag="rs4",
                                      name=f"rs4{h}")
                    rc4 = smalls.tile([128, 4], F32, tag="rc4",
                                      name=f"rc4{h}")
                    nc.vector.tensor_copy(onum[h * 64:(h + 1) * 64, :],
                                          o_ps[h][0:64, :])
                    nc.vector.tensor_copy(rec[64:65, :], o_ps[h][64:65, :])
                    # reshape across 128 partitions so the iterative-divide
                    # reciprocal runs 128 lanes wide
                    nc.sync.dma_start(rs4[:], rec[64:65, :])
                    nc.vector.reciprocal(rc4[:], rs4[:])
                    nc.sync.dma_start(rec[64:65, :], rc4[:])
                    nc.vector.tensor_copy(recb[64:65, :], rec[64:65, :])
                    # per-head broadcast into its 64-row half of one psum bank
                    nc.tensor.matmul(
                        bc_ps[h * 64:(h + 1) * 64, :],
                        ones_sb[64:65, 0:64],
                        recb[64:65, :],
                    )
                bc_sb = stage.tile([128, 512], F32, tag="bc_sb")
                nc.vector.tensor_copy(bc_sb[:], bc_ps[:])
                nc.vector.tensor_mul(outT[:, qsl], onum[:], bc_sb[:])

                # out_proj for this q-chunk (4 s-chunks of 128)
                for si in range(4):
                    sc0 = qs + si * 128
                    for n in range(2):
                        yp = psY.tile([128, 512], F32, tag="yp")
                        nc.tensor.matmul(
                            yp[:],
                            outT[:, sc0:sc0 + 128],
                            wo_sb[:, n * 512:(n + 1) * 512],
                        )
                        ysb = ysb_p.tile([128, 512], F32, tag="ysb")
                        nc.vector.tensor_copy(ysb[:], yp[:])
                        # bulk stores ride the SWDGE queues so the tiny
                        # latency-critical reciprocal DMAs keep the sync
                        # queues to themselves
                        nc.gpsimd.dma_start(
                            y[sc0:sc0 + 128, n * 512:(n + 1) * 512], ysb[:]
                        )

    nc.compile()
    return nc


# ---------------------------------------------------------------------------
# entry point
# ---------------------------------------------------------------------------

_PROGRAM_CACHE = {}
TRACE = False
LAST_RESULTS = None


def _get_program(s_len):
    if s_len not in _PROGRAM_CACHE:
        _PROGRAM_CACHE[s_len] = build_program(s_len)
    return _PROGRAM_CACHE[s_len]


def kernel(x, Wq, Wk, Wv, Wo):
    global LAST_RESULTS
    from concourse.bass_utils import run_bass_kernel_spmd

    x = np.asarray(x)
    b, s_len, _ = x.shape
    per_core = shard_inputs(x, Wq, Wk, Wv, Wo)
    nc = _get_program(s_len)
    res = run_bass_kernel_spmd(
        nc, per_core, core_ids=list(range(N_CORES)), trace=TRACE
    )
    LAST_RESULTS = res
    y = np.zeros((s_len, D_MODEL), np.float32)
    for r in res.results:
        y += r["y"]
    return y.reshape(b, s_len, D_MODEL)


# revision 27
# speedup vs baseline: 1.0590x; 1.0590x over previous
"""Multi-head self-attention with RoPE on 8 Trainium2 NeuronCores.

Tensor-parallel over heads: core c owns heads 2c, 2c+1.  Each core computes
Q/K/V projections for its heads (full sequence), causal attention in a
transposed layout, and a partial out-projection against its 128-column slice
of Wo.  The host sums the 8 partial outputs.

Matmuls run in bf16 (fp32 PSUM accumulation): bf16 gets fast weight load and
weight-load/matmul overlap on the PE, which fp32/fp32r do not.

Per-core dataflow (S=4096, D=1024, hd=64):
  xT [1024,4096](bf16) --PE--> qt/kt [128,4096] via RoPE, vt [128,4096]
  RoPE: qt = ps*cs + swap32(ps*sn_signed); the 32-row swap runs on idle DMA
  engines, channel order pre-permuted on the host ([evens|odds] per head)
  vt --PE transpose--> vaug[h] [128, 32*65] bf16 (V natural layout + ones
  column per 128-k-chunk; the ones column is the softmax denominator)
  scores^T [k,q] = kt_h^T @ qt_h (K=64) in PSUM; causal diagonal tiles get a
  [128,128] additive band mask; exp on ScalarE (scale=1/8 folded in) writes
  bf16 probs, skipping fully-masked columns
  attn@V: vaug [128,65] x probs [128,512] accumulated over k-chunks in PSUM
  normalize: approx reciprocal of row 64, PE K=1 broadcast, DVE multiply
  out_proj: outT [128,128] x woT [128,512], DVE evict, DMA to DRAM
"""

from contextlib import ExitStack

import ml_dtypes
import numpy as np

import concourse.bass as bass
import concourse.mybir as mybir
import concourse.tile as tile
from concourse import bacc
from concourse.masks import make_identity

F32 = mybir.dt.float32
BF16 = mybir.dt.bfloat16
EXPF = mybir.ActivationFunctionType.Exp
COPYF = mybir.ActivationFunctionType.Copy

D_MODEL = 1024
HEAD_DIM = 64
HALF = 32
THETA = 10000.0
N_CORES = 8
HPC = 2  # heads per core


# ---------------------------------------------------------------------------
# host-side sharding
# ---------------------------------------------------------------------------

def _perm128():
    # within each head's 64 rows: evens then odds, so RoPE pairs are p <-> p+-32
    g = []
    for h in range(HPC):
        for par in (0, 1):
            g.extend(h * HEAD_DIM + 2 * i + par for i in range(HALF))
    return np.array(g, dtype=np.int64)


def _rope_tables(s_len):
    # qt = ps*cs + swap32(ps*sn), where swap32 exchanges rows p <-> p+-32
    # within each 64-row head block.  Row p of sn carries the sign that the
    # OUTPUT row swap(p) needs: +sin for p%64<32, -sin otherwise.
    inv_freq = 1.0 / (THETA ** (np.arange(HALF, dtype=np.float64) * 2.0 / HEAD_DIM))
    ang = np.arange(s_len, dtype=np.float64)[None, :] * inv_freq[:, None]
    cs = np.tile(np.cos(ang), (4, 1)).astype(np.float32)
    sinv = np.sin(ang)
    sn = np.concatenate([sinv, -sinv, sinv, -sinv], axis=0).astype(np.float32)
    return cs, sn


def _band_mask():
    # additive diagonal band: band[kk, b] = 0 if b >= kk else -240
    kk = np.arange(128)[:, None]
    b = np.arange(128)[None, :]
    return np.where(b >= kk, 0.0, -240.0).astype(np.float32)


def _bf16(a):
    return np.asarray(a, np.float32).astype(ml_dtypes.bfloat16)


def shard_inputs(x, Wq, Wk, Wv, Wo):
    x2 = np.ascontiguousarray(np.asarray(x, np.float32).reshape(-1, D_MODEL))
    s_len = x2.shape[0]
    xT = np.ascontiguousarray(_bf16(x2.T))
    cs, sn = _rope_tables(s_len)
    band = _band_mask()
    g = _perm128()
    Wq, Wk, Wv, Wo = (np.asarray(w, np.float32) for w in (Wq, Wk, Wv, Wo))
    per_core = []
    for c in range(N_CORES):
        rows = slice(c * 128, (c + 1) * 128)
        per_core.append({
            "xT": xT,
            "wqT": np.ascontiguousarray(_bf16(Wq[rows][g].T)),
            "wkT": np.ascontiguousarray(_bf16(Wk[rows][g].T)),
            "wvT": np.ascontiguousarray(_bf16(Wv[rows].T)),
            "woT": np.ascontiguousarray(_bf16(Wo[:, rows].T)),
            "cs": _bf16(cs),
            "sn": _bf16(sn),
            "band": band,
        })
    return per_core


# ---------------------------------------------------------------------------
# device kernel
# ---------------------------------------------------------------------------

def build_program(s_len):
    """Build the SPMD Bass program for one core (same program, all cores)."""
    n_sc = s_len // 512   # 512-wide s/q chunks
    n_kc = s_len // 128   # 128-wide k chunks
    n_dc = D_MODEL // 128

    nc = bacc.Bacc("TRN2", target_bir_lowering=False, debug=False)

    xT = nc.dram_tensor("xT", [D_MODEL, s_len], BF16, kind="ExternalInput").ap()
    wqT = nc.dram_tensor("wqT", [D_MODEL, 128], BF16, kind="ExternalInput").ap()
    wkT = nc.dram_tensor("wkT", [D_MODEL, 128], BF16, kind="ExternalInput").ap()
    wvT = nc.dram_tensor("wvT", [D_MODEL, 128], BF16, kind="ExternalInput").ap()
    woT = nc.dram_tensor("woT", [128, D_MODEL], BF16, kind="ExternalInput").ap()
    cs_d = nc.dram_tensor("cs", [128, s_len], BF16, kind="ExternalInput").ap()
    sn_d = nc.dram_tensor("sn", [128, s_len], BF16, kind="ExternalInput").ap()
    band_d = nc.dram_tensor("band", [128, 128], F32, kind="ExternalInput").ap()
    y = nc.dram_tensor("y", [s_len, D_MODEL], F32, kind="ExternalOutput").ap()

    with tile.TileContext(nc) as tc, ExitStack() as ctx:
        const = ctx.enter_context(tc.tile_pool(name="const", bufs=1))
        resid = ctx.enter_context(tc.tile_pool(name="resid", bufs=1))
        xin = ctx.enter_context(tc.tile_pool(name="xin", bufs=32))
        stage = ctx.enter_context(tc.tile_pool(name="stage", bufs=4))
        probs_p = ctx.enter_context(tc.tile_pool(name="probs", bufs=8))
        ysb_p = ctx.enter_context(tc.tile_pool(name="ysb", bufs=4))
        smalls = ctx.enter_context(tc.tile_pool(name="smalls", bufs=2))

        # ---- constants / weights in SBUF ----
        wq_sb = const.tile([128, D_MODEL], BF16, tag="wq")
        wk_sb = const.tile([128, D_MODEL], BF16, tag="wk")
        wv_sb = const.tile([128, D_MODEL], BF16, tag="wv")
        for d in range(n_dc):
            sl = slice(d * 128, (d + 1) * 128)
            nc.gpsimd.dma_start(wq_sb[:, sl], wqT[sl, :])
            nc.gpsimd.dma_start(wk_sb[:, sl], wkT[sl, :])
            nc.gpsimd.dma_start(wv_sb[:, sl], wvT[sl, :])
        wo_sb = const.tile([128, D_MODEL], BF16, tag="wo")
        nc.gpsimd.dma_start(wo_sb[:], woT[:])
        cs_sb = const.tile([128, s_len], BF16, tag="cs")
        sn_sb = const.tile([128, s_len], BF16, tag="sn")
        nc.gpsimd.dma_start(cs_sb[:], cs_d[:])
        nc.gpsimd.dma_start(sn_sb[:], sn_d[:])
        band_sb = const.tile([128, 128], F32, tag="band")
        nc.gpsimd.dma_start(band_sb[:], band_d[:])
        ident = const.tile([128, 128], BF16, tag="ident")
        make_identity(nc, ident[:])
        ones_sb = const.tile([65, 128], BF16, tag="ones")
        nc.vector.memset(ones_sb[:], 1.0)
        zbias = const.tile([128, 1], F32, tag="zbias")
        nc.vector.memset(zbias[:], 0.0)

        # ---- persistent activations ----
        qt = resid.tile([128, s_len], BF16, tag="qt")
        kt = resid.tile([128, s_len], BF16, tag="kt")
        vt = resid.tile([128, s_len], BF16, tag="vt")
        vaug = [resid.tile([128, n_kc * 65], BF16, tag=f"vaug{h}", name=f"vaug{h}")
                for h in range(HPC)]
        outT = resid.tile([128, s_len], BF16, tag="outT")
        for h in range(HPC):
            ones_cols = vaug[h].rearrange("p (k c) -> p k c", c=65)[:, :, 64:65]
            nc.vector.memset(ones_cols, 1.0)

        # ================= phase A: QKV + RoPE + V transpose =================
        with tc.tile_pool(name="psA", bufs=2, space="PSUM") as psA:
            for j in range(n_sc):
                jsl = slice(j * 512, (j + 1) * 512)
                xts = []
                for d in range(n_dc):
                    xt = xin.tile([128, 512], BF16, tag="xt")
                    nc.sync.dma_start(xt[:], xT[d * 128:(d + 1) * 128, jsl])
                    xts.append(xt)

                ps_q = psA.tile([128, 512], F32, tag="q")
                ps_k = psA.tile([128, 512], F32, tag="k")
                ps_v = psA.tile([128, 512], F32, tag="v")
                for w_sb, ps in ((wq_sb, ps_q), (wk_sb, ps_k), (wv_sb, ps_v)):
                    for d in range(n_dc):
                        nc.tensor.matmul(
                            ps[:],
                            w_sb[:, d * 128:(d + 1) * 128],
                            xts[d][:],
                            start=(d == 0),
                            stop=(d == n_dc - 1),
                        )

                # RoPE: qt = ps*cs + swap32(ps*sn_signed).  The 32-row swap
                # runs on idle DMA engines (SBUF->SBUF); the final add sees
                # base-partition-matched SBUF operands (a DVE requirement).
                for ps, dst in ((ps_q, qt), (ps_k, kt)):
                    P = stage.tile([128, 512], F32, tag="P")
                    Rr = stage.tile([128, 512], F32, tag="Rr")
                    Rs = stage.tile([128, 512], F32, tag="Rs")
                    nc.vector.tensor_mul(P[:], ps[:], cs_sb[:, jsl])
                    nc.vector.tensor_mul(Rr[:], ps[:], sn_sb[:, jsl])
                    for q in range(4):
                        src = q + 1 if q % 2 == 0 else q - 1
                        nc.sync.dma_start(Rs[q * 32:(q + 1) * 32, :],
                                          Rr[src * 32:(src + 1) * 32, :])
                    nc.vector.tensor_add(dst[:, jsl], P[:], Rs[:])
                nc.scalar.copy(vt[:, jsl], ps_v[:])

                # V transpose: 4 k-chunks per s-chunk, natural layout + ones col
                for kk in range(4):
                    kc = 4 * j + kk
                    pt = psA.tile([128, 128], BF16, tag="pt")
                    nc.tensor.transpose(
                        pt[:], vt[:, kc * 128:(kc + 1) * 128], ident[:]
                    )
                    for h in range(HPC):
                        nc.vector.tensor_copy(
                            vaug[h][:, kc * 65:kc * 65 + 64],
                            pt[:, h * 64:(h + 1) * 64],
                        )

        # ================= phase B: attention + out_proj =================
        with (
            tc.tile_pool(name="psS", bufs=2, space="PSUM") as psS,
            tc.tile_pool(name="psO", bufs=1, space="PSUM") as psO,
            tc.tile_pool(name="psY", bufs=1, space="PSUM") as psY,
        ):
            for qc in range(n_sc):
                qs = qc * 512
                qsl = slice(qs, qs + 512)
                nk = 4 * (qc + 1)
                o_ps = [psO.tile([65, 512], F32, tag=f"o{h}", name=f"o{h}")
                        for h in range(HPC)]
                for pair in range(nk // 2):
                    kcs = (2 * pair, 2 * pair + 1)
                    for h in range(HPC):
                        rb = 64 * h
                        sc_ps = psS.tile([128, 1024], F32, tag="sc")
                        for side, kc in enumerate(kcs):
                            ssl = slice(side * 512, side * 512 + 512)
                            nc.tensor.matmul(
                                sc_ps[:, ssl],
                                kt[rb:rb + 64, kc * 128:(kc + 1) * 128],
                                qt[rb:rb + 64, qsl],
                            )
                        pr = probs_p.tile([128, 1024], BF16, tag="pr")
                        diag = [kc * 128 - qs for kc in kcs]
                        if diag[1] < 0:
                            # fully-valid pair: one wide exp
                            nc.scalar.activation(pr[:], sc_ps[:], EXPF,
                                                 bias=zbias[:], scale=0.125)
                        else:
                            for side, kc in enumerate(kcs):
                                r = diag[side]
                                c0 = side * 512
                                if r < 0:
                                    nc.scalar.activation(
                                        pr[:, c0:c0 + 512],
                                        sc_ps[:, c0:c0 + 512],
                                        EXPF, bias=zbias[:], scale=0.125)
                                    continue
                                # columns < r are fully masked; the 128-wide
                                # band at [r, r+128) gets the additive mask
                                nc.vector.tensor_add(
                                    sc_ps[:, c0 + r:c0 + r + 128],
                                    sc_ps[:, c0 + r:c0 + r + 128],
                                    band_sb[:],
                                )
                                if r > 0:
                                    nc.vector.memset(pr[:, c0:c0 + r], 0.0)
                                nc.scalar.activation(
                                    pr[:, c0 + r:c0 + 512],
                                    sc_ps[:, c0 + r:c0 + 512],
                                    EXPF, bias=zbias[:], scale=0.125)
                        for side, kc in enumerate(kcs):
                            ssl = slice(side * 512, side * 512 + 512)
                            nc.tensor.matmul(
                                o_ps[h][:],
                                vaug[h][:, kc * 65:kc * 65 + 65],
                                pr[:, ssl],
                                start=(kc == 0),
                                stop=(kc == nk - 1),
                            )

                # normalize: outT[h] = o_ps[h][0:64] * (1/denom) broadcast.
                # Evict the accumulator to SBUF with two fast copies first so
                # the PSUM bank frees in ~1.3us; the reciprocal/broadcast
                # chain then runs off the critical path (its latency was
                # stalling the next q-chunk's attn@V and re-throttling HAM).
                onum = smalls.tile([128, 512], F32, tag="onum")
                bc_ps = psY.tile([128, 512], F32, tag="bc")
                for h in range(HPC):
                    rec = smalls.tile([65, 512], F32, tag="rec",
                                      name=f"rec{h}")
                    recb = smalls.tile([65, 512], BF16, tag="recb",
                                       name=f"recb{h}")
                    rs4 = smalls.tile([128, 4], F32, tag="rs4",
                                      name=f"rs4{h}")
                    rc4 = smalls.tile([128, 4], F32, tag="rc4",
                                      name=f"rc4{h}")
                    nc.vector.tensor_copy(onum[h * 64:(h + 1) * 64, :],
                                          o_ps[h][0:64, :])
                    nc.vector.tensor_copy(rec[64:65, :], o_ps[h][64:65, :])
                    # reshape across 128 partitions so the iterative-divide
                    # reciprocal runs 128 lanes wide
                    nc.sync.dma_start(rs4[:], rec[64:65, :])
                    nc.vector.reciprocal(rc4[:], rs4[:])
                    nc.sync.dma_start(rec[64:65, :], rc4[:])
                    nc.vector.tensor_copy(recb[64:65, :], rec[64:65, :])
                    # per-head broadcast into its 64-row half of one psum bank
                    nc.tensor.matmul(
                        bc_ps[h * 64:(h + 1) * 64, :],
                        ones_sb[64:65, 0:64],
                        recb[64:65, :],
                    )
                bc_sb = stage.tile([128, 512], F32, tag="bc_sb")
                nc.vector.tensor_copy(bc_sb[:], bc_ps[:])
                nc.vector.tensor_mul(outT[:, qsl], onum[:], bc_sb[:])

                # out_proj for this q-chunk (4 s-chunks of 128)
                for si in range(4):
                    sc0 = qs + si * 128
                    for n in range(2):
                        yp = psY.tile([128, 512], F32, tag="yp")
                        nc.tensor.matmul(
                            yp[:],
                            outT[:, sc0:sc0 + 128],
                            wo_sb[:, n * 512:(n + 1) * 512],
                        )
                        ysb = ysb_p.tile([128, 512], F32, tag="ysb")
                        nc.vector.tensor_copy(ysb[:], yp[:])
                        nc.sync.dma_start(
                            y[sc0:sc0 + 128, n * 512:(n + 1) * 512], ysb[:]
                        )

    nc.compile()
    return nc


# ---------------------------------------------------------------------------
# entry point
# ---------------------------------------------------------------------------

_PROGRAM_CACHE = {}
TRACE = False
LAST_RESULTS = None


def _get_program(s_len):
    if s_len not in _PROGRAM_CACHE:
        _PROGRAM_CACHE[s_len] = build_program(s_len)
    return _PROGRAM_CACHE[s_len]


def kernel(x, Wq, Wk, Wv, Wo):
    global LAST_RESULTS
    from concourse.bass_utils import run_bass_kernel_spmd

    x = np.asarray(x)
    b, s_len, _ = x.shape
    per_core = shard_inputs(x, Wq, Wk, Wv, Wo)
    nc = _get_program(s_len)
    res = run_bass_kernel_spmd(
        nc, per_core, core_ids=list(range(N_CORES)), trace=TRACE
    )
    LAST_RESULTS = res
    y = np.zeros((s_len, D_MODEL), np.float32)
    for r in res.results:
        y += r["y"]
    return y.reshape(b, s_len, D_MODEL)
